# revision 6
# baseline (speedup 1.0000x reference)
"""GNN message-passing kernel for 8 Trainium2 NeuronCores.

Strategy: dst-partition nodes 8 ways (12544/core incl pad). Per GraphConv
layer (one SPMD launch; layers 1-2 share a NEFF, layer 3 adds pooling):
  A) each core computes z = h @ W for ALL nodes (replicated, bf16 PE work),
     stores z bf16 in 4 per-window HBM tensors (windows keep gather idxs
     in int16 range).
  B) per-edge messages gathered via Q7 dma_gather (int16 idxs), edges
     pre-sorted by (window, dst_tile) into static slot groups (max over
     cores, shared NEFF schedule). Gather calls round-robin across the
     4 windows / swdge queues so consumption never deadlocks.
  C) segmented reduction on the PE: per 128-edge chunk a one-hot S matrix
     scaled by nsrc[src] (built on DVE via a single fused
     tensor_scalar(is_equal, mult) against an iota tile); PSUM accumulates
     all chunks of a dst tile (tile-major schedule, no SBUF A-tiles).
  D) h' = relu(psum * ndst + b): one DVE scalar_tensor_tensor + one ACT
     relu per tile; layer-3 NEFF also accumulates per-graph pool partials
     poolT[f,g] += hn^T @ onehot(gid) on two persistent PSUM banks.
A tiny 4th launch sums the 8 cores' pool partials, applies 1/count and the
MLP tail (replicated on all cores). All float math on x runs on device; the
host only computes integer edge/group structure and degree norms
(graph-structure metadata) and reshapes/casts activations between launches.
"""
import sys, types, os
sys.path.insert(0, "/opt/trn_rl_repo")

try:
    import antenv.axon_hooks  # noqa: F401
except Exception:
    try:
        import antenv
        from trn_agent_boot.trn_boot import _ntff_profile_via_ctypes
        _hook = _ntff_profile_via_ctypes("/opt/axon/libaxon_pjrt.so")
        _m = types.ModuleType("antenv.axon_hooks")
        _m.get_axon_ntff_profile_hook = lambda: _hook
        _m.set_axon_ntff_profile_hook = lambda h: None
        sys.modules["antenv.axon_hooks"] = _m
        antenv.axon_hooks = _m
    except Exception:
        pass

import numpy as np
import ml_dtypes
import concourse.bacc as bacc
import concourse.mybir as mybir
import concourse.tile as tile
from concourse.bass_utils import run_bass_kernel_spmd

P = 128
N_NODES, N_EDGES, N_GRAPHS = 100000, 1600000, 256
D = 128
NC = 8
OWN = 12544                    # dst nodes per core (incl pad on core 7)
NT = OWN // P                  # 98 dst tiles per core
NW = 4                         # z windows (int16 gather idx range)
WIN = 25088                    # rows per window
PADN = NW * WIN                # 100352 padded node rows
NZC = PADN // P                # 784 z chunks
GCALL = 4096                   # max edges per dma_gather call
NOMATCH = 240.0                # dl value that never matches iota 0..127

LAST_EXEC_NS = []


def _pack_idxs(idx):
    n = len(idx)
    S = (n + 15) // 16
    arr = np.zeros((16, S), dtype=np.int16)
    arr[np.arange(n) % 16, np.arange(n) // 16] = idx.astype(np.int16)
    return np.tile(arr, (8, 1))


def _prep(edge_src, edge_dst, node2graph):
    es = np.asarray(edge_src).astype(np.int64)
    ed = np.asarray(edge_dst).astype(np.int64)
    n2g = np.asarray(node2graph)
    out_deg = np.bincount(es, minlength=N_NODES).astype(np.float32)
    in_deg = np.bincount(ed, minlength=N_NODES).astype(np.float32)
    nsrc = 1.0 / np.sqrt(np.maximum(out_deg, 1.0))
    ndst = 1.0 / np.sqrt(np.maximum(in_deg, 1.0))

    NG = NW * NT  # 392 groups
    cnts = np.zeros((NC, NG), np.int64)
    core_data = []
    for c in range(NC):
        m = (ed // OWN) == c
        s, d = es[m], ed[m]
        dl = d - OWN * c
        t = dl >> 7
        w = s // WIN
        key = w * NT + t
        order = np.argsort(key, kind="stable")
        cnts[c] = np.bincount(key, minlength=NG)
        core_data.append((s[order], dl[order], key[order]))

    slots_g = (((cnts.max(axis=0) + P - 1) // P) * P).astype(np.int64)  # [392]
    chunks_g = slots_g // P
    group_start = np.zeros(NG + 1, np.int64)
    group_start[1:] = np.cumsum(slots_g)
    tot_slots = int(group_start[-1])
    tot_chunks = tot_slots // P

    # per-stream static call plan (streams are contiguous group ranges)
    stream_slots = [int(slots_g[w * NT:(w + 1) * NT].sum()) for w in range(NW)]
    stream_base = np.zeros(NW + 1, np.int64)
    stream_base[1:] = np.cumsum(stream_slots)
    calls = []           # [(w, n_slots), ...] in round-robin issue order
    calls_per_w = []
    for w in range(NW):
        r, lst = stream_slots[w], []
        while r > 0:
            n = min(GCALL, r)
            lst.append(n)
            r -= n
        calls_per_w.append(lst)
    ncall_max = max(len(l) for l in calls_per_w)
    for ci in range(ncall_max):
        for w in range(NW):
            if ci < len(calls_per_w[w]):
                calls.append((w, calls_per_w[w][ci]))

    # chunk -> (stream call index, slot in call) static map
    chunk_map = {}
    for w in range(NW):
        off = 0
        for ci, n in enumerate(calls_per_w[w]):
            for k in range(n // P):
                chunk_map[(w, off // P + k)] = (ci, k)
            off += n

    per_core = []
    for c in range(NC):
        s, dl, key = core_data[c]
        idx_stream = np.zeros(tot_slots, np.int64)
        dl_stream = np.full(tot_slots, NOMATCH, np.float64)
        ns_stream = np.zeros(tot_slots, np.float64)
        if len(key):
            gidx = group_start[key] + np.concatenate(
                [np.arange(n) for n in np.bincount(key, minlength=NG)])
            idx_stream[gidx] = s % WIN
            dl_stream[gidx] = dl & 127
            ns_stream[gidx] = nsrc[s]
        packs = [_pack_idxs(idx_stream[stream_base[w]:stream_base[w + 1]])
                 for w in range(NW)]
        idx16 = np.concatenate(packs, axis=1)
        dlc = dl_stream.reshape(tot_chunks, P).T.astype(np.float32).copy()
        nsl = ns_stream.reshape(tot_chunks, P).T.astype(np.float32).copy()

        gid = np.full(OWN, -1.0, np.float32)
        lo, hi = c * OWN, min((c + 1) * OWN, N_NODES)
        gid[:hi - lo] = n2g[lo:hi]
        nd = np.zeros(OWN, np.float32)
        nd[:hi - lo] = ndst[lo:hi]
        per_core.append(dict(
            idx16=idx16, dl=dlc, nsl=nsl,
            ndstc=nd.reshape(NT, P).T.copy(),
            gidc=gid.reshape(NT, P).T.copy()))

    cnt = np.bincount(n2g, minlength=N_GRAPHS).astype(np.float32)
    inv_cnt = (1.0 / np.maximum(cnt, 1.0))

    meta = dict(slots_g=slots_g, chunks_g=chunks_g, tot_slots=tot_slots,
                tot_chunks=tot_chunks, calls=calls, calls_per_w=calls_per_w,
                chunk_map=chunk_map)
    return per_core, meta, inv_cnt


def _build_conv(meta, with_pool):
    chunks_g = meta["chunks_g"]
    tot_chunks = meta["tot_chunks"]
    tot_slots = meta["tot_slots"]
    calls = meta["calls"]
    calls_per_w = meta["calls_per_w"]
    chunk_map = meta["chunk_map"]
    IDXC = tot_slots // 16

    nc = bacc.Bacc("TRN2", num_devices=NC, num_swdge_queues=4)
    hT = nc.dram_tensor("hT", [P, PADN], mybir.dt.bfloat16, kind="ExternalInput")
    W = nc.dram_tensor("W", [D, D], mybir.dt.bfloat16, kind="ExternalInput")
    brep = nc.dram_tensor("brep", [P, D], mybir.dt.float32, kind="ExternalInput")
    ndstc = nc.dram_tensor("ndstc", [P, NT], mybir.dt.float32, kind="ExternalInput")
    dl = nc.dram_tensor("dl", [P, tot_chunks], mybir.dt.float32,
                        kind="ExternalInput")
    nsl = nc.dram_tensor("nsl", [P, tot_chunks], mybir.dt.float32,
                         kind="ExternalInput")
    idx16 = nc.dram_tensor("idx16", [P, IDXC], mybir.dt.int16, kind="ExternalInput")
    iota = nc.dram_tensor("iota", [P, P], mybir.dt.bfloat16, kind="ExternalInput")
    if with_pool:
        gidc = nc.dram_tensor("gidc", [P, NT], mybir.dt.float32,
                              kind="ExternalInput")
        iotaB = nc.dram_tensor("iotaB", [P, P], mybir.dt.bfloat16,
                               kind="ExternalInput")
        poolT = nc.dram_tensor("poolT", [P, N_GRAPHS], mybir.dt.float32,
                               kind="ExternalOutput")
    else:
        hout = nc.dram_tensor("hout", [OWN, D], mybir.dt.bfloat16,
                              kind="ExternalOutput")
    zw = [nc.dram_tensor(f"z{w}", [WIN, D], mybir.dt.bfloat16) for w in range(NW)]

    with tile.TileContext(nc) as tc:
        with tc.tile_pool(name="const", bufs=1) as cp, \
             tc.tile_pool(name="hblk", bufs=2) as hp, \
             tc.tile_pool(name="zst", bufs=3) as zp, \
             tc.tile_pool(name="zps", bufs=2, space="PSUM") as zps, \
             tc.tile_pool(name="m0", bufs=2) as mp0, \
             tc.tile_pool(name="m1", bufs=2) as mp1, \
             tc.tile_pool(name="m2", bufs=2) as mp2, \
             tc.tile_pool(name="m3", bufs=2) as mp3, \
             tc.tile_pool(name="smat", bufs=6) as sp, \
             tc.tile_pool(name="cps", bufs=3, space="PSUM") as cpsp, \
             tc.tile_pool(name="pps", bufs=1, space="PSUM") as ppsp, \
             tc.tile_pool(name="dph", bufs=3) as dp:
            mpools = [mp0, mp1, mp2, mp3]
            W_sb = cp.tile([D, D], mybir.dt.bfloat16, tag="W")
            nc.sync.dma_start(out=W_sb[:], in_=W[:])
            brep_sb = cp.tile([P, D], mybir.dt.float32, tag="brep")
            nc.sync.dma_start(out=brep_sb[:], in_=brep[:])
            ndst_sb = cp.tile([P, NT], mybir.dt.float32, tag="ndst")
            nc.sync.dma_start(out=ndst_sb[:], in_=ndstc[:])
            dl_sb = cp.tile([P, tot_chunks], mybir.dt.float32, tag="dl")
            nc.sync.dma_start(out=dl_sb[:], in_=dl[:])
            nsl_sb = cp.tile([P, tot_chunks], mybir.dt.float32, tag="nsl")
            nc.sync.dma_start(out=nsl_sb[:], in_=nsl[:])
            idx_sb = cp.tile([P, IDXC], mybir.dt.int16, tag="idx")
            nc.sync.dma_start(out=idx_sb[:], in_=idx16[:])
            iota_sb = cp.tile([P, P], mybir.dt.bfloat16, tag="iota")
            nc.sync.dma_start(out=iota_sb[:], in_=iota[:])
            if with_pool:
                gid_sb = cp.tile([P, NT], mybir.dt.float32, tag="gid")
                nc.sync.dma_start(out=gid_sb[:], in_=gidc[:])
                iotaB_sb = cp.tile([P, P], mybir.dt.bfloat16, tag="iotaB")
                nc.sync.dma_start(out=iotaB_sb[:], in_=iotaB[:])
                pool_ps0 = ppsp.tile([P, P], mybir.dt.float32, tag="pool0")
                pool_ps1 = ppsp.tile([P, P], mybir.dt.float32, tag="pool1")
                pool_ps = [pool_ps0, pool_ps1]

            # ---- phase A: z = h @ W  (bf16), per window ----
            ZB = 4   # chunks per psum bank / staging store
            HB = 28  # chunks per hT staging block (28*4 chunks per window)
            WCH = WIN // P  # 196 chunks per window
            for w in range(NW):
                zv = zw[w][:].rearrange("(a k n) f -> a n k f", n=P, k=ZB)
                for blk in range(WCH // HB):
                    c0 = blk * HB
                    hT_sb = hp.tile([P, HB * P], mybir.dt.bfloat16, tag="h")
                    nc.sync.dma_start(
                        out=hT_sb[:],
                        in_=hT[:, w * WIN + c0 * P:w * WIN + (c0 + HB) * P])
                    for g0 in range(0, HB, ZB):
                        ps = zps.tile([P, ZB, D], mybir.dt.float32, tag="zps")
                        for k in range(ZB):
                            nc.tensor.matmul(
                                out=ps[:, k, :],
                                lhsT=hT_sb[:, (g0 + k) * P:(g0 + k + 1) * P],
                                rhs=W_sb[:], start=True, stop=True)
                        zst = zp.tile([P, ZB, D], mybir.dt.bfloat16, tag="zst")
                        nc.scalar.activation(
                            out=zst[:], in_=ps[:],
                            func=mybir.ActivationFunctionType.Copy)
                        nc.sync.dma_start(out=zv[(c0 + g0) // ZB], in_=zst[:])

            # ---- phase B: gathers, round-robin across windows/queues ----
            msg_tiles = {w: [] for w in range(NW)}
            woff = [0] * NW
            for (w, n) in calls:
                mt = mpools[w].tile([P, n // P, D], mybir.dt.bfloat16,
                                    tag=f"msg{w}")
                nc.gpsimd.dma_gather(
                    mt[:], zw[w][:],
                    idx_sb[:, (meta_sb_off(meta, w) + woff[w]) // 16:
                           (meta_sb_off(meta, w) + woff[w] + n) // 16],
                    n, n, D, queue_num=w, single_packet=False)
                msg_tiles[w].append(mt)
                woff[w] += n

            # ---- phase C+D: tile-major segmented reduce ----
            gchunk_base = np.zeros(NW, np.int64)
            off = 0
            for w in range(NW):
                gchunk_base[w] = off
                off += sum(calls_per_w[w]) // P
            group_chunk_start = {}
            for w in range(NW):
                off = 0
                for t in range(NT):
                    group_chunk_start[(w, t)] = off
                    off += int(chunks_g[w * NT + t])

            for t in range(NT):
                jobs = []
                for w in range(NW):
                    for k in range(int(chunks_g[w * NT + t])):
                        cw = group_chunk_start[(w, t)] + k
                        jobs.append((w, cw))
                ps = cpsp.tile([P, D], mybir.dt.float32, tag="cps")
                for ji, (w, cw) in enumerate(jobs):
                    gc = int(gchunk_base[w]) + cw
                    S = sp.tile([P, P], mybir.dt.bfloat16, tag="S")
                    nc.vector.tensor_scalar(
                        out=S[:], in0=iota_sb[:],
                        scalar1=dl_sb[:, gc:gc + 1],
                        scalar2=nsl_sb[:, gc:gc + 1],
                        op0=mybir.AluOpType.is_equal,
                        op1=mybir.AluOpType.mult)
                    ci, slot = chunk_map[(w, cw)]
                    nc.tensor.matmul(
                        out=ps[:], lhsT=S[:], rhs=msg_tiles[w][ci][:, slot, :],
                        start=(ji == 0), stop=(ji == len(jobs) - 1))
                hn = dp.tile([P, D], mybir.dt.bfloat16, tag="hn")
                nc.vector.scalar_tensor_tensor(
                    out=hn[:], in0=ps[:], scalar=ndst_sb[:, t:t + 1],
                    in1=brep_sb[:], op0=mybir.AluOpType.mult,
                    op1=mybir.AluOpType.add)
                nc.scalar.activation(out=hn[:], in_=hn[:],
                                     func=mybir.ActivationFunctionType.Relu)
                if with_pool:
                    for b, io_sb in enumerate((iota_sb, iotaB_sb)):
                        Sg = sp.tile([P, P], mybir.dt.bfloat16, tag="S")
                        nc.vector.tensor_single_scalar(
                            out=Sg[:], in_=io_sb[:],
                            scalar=gid_sb[:, t:t + 1],
                            op=mybir.AluOpType.is_equal)
                        nc.tensor.matmul(
                            out=pool_ps[b][:], lhsT=hn[:], rhs=Sg[:],
                            start=(t == 0), stop=(t == NT - 1))
                else:
                    nc.sync.dma_start(out=hout[t * P:(t + 1) * P, :], in_=hn[:])

            if with_pool:
                pool_sb = dp.tile([P, 2, P], mybir.dt.float32, tag="poolsb")
                for b in range(2):
                    nc.vector.tensor_copy(out=pool_sb[:, b, :], in_=pool_ps[b][:])
                nc.sync.dma_start(
                    out=poolT[:], in_=pool_sb[:].rearrange("p b g -> p (b g)"))
    nc.compile()
    return nc


def meta_sb_off(meta, w):
    """Slot offset of stream w in the concatenated idx16 layout."""
    off = 0
    for ww in range(w):
        off += sum(meta["calls_per_w"][ww])
    return off


def _build_tail():
    nc = bacc.Bacc("TRN2", num_devices=NC, num_swdge_queues=1)
    pall = nc.dram_tensor("pall", [NC * P, N_GRAPHS], mybir.dt.float32,
                          kind="ExternalInput")
    invc = nc.dram_tensor("invc", [P, N_GRAPHS], mybir.dt.float32,
                          kind="ExternalInput")
    Wf0 = nc.dram_tensor("Wf0", [128, 256], mybir.dt.float32, kind="ExternalInput")
    bf0 = nc.dram_tensor("bf0", [256, 1], mybir.dt.float32, kind="ExternalInput")
    Wf1 = nc.dram_tensor("Wf1", [256, 256], mybir.dt.float32, kind="ExternalInput")
    bf1 = nc.dram_tensor("bf1", [256, 1], mybir.dt.float32, kind="ExternalInput")
    Wout = nc.dram_tensor("Wout", [256, 8], mybir.dt.float32, kind="ExternalInput")
    bout = nc.dram_tensor("bout", [8, 1], mybir.dt.float32, kind="ExternalInput")
    outT = nc.dram_tensor("outT", [8, N_GRAPHS], mybir.dt.float32,
                          kind="ExternalOutput")

    with tile.TileContext(nc) as tc:
        with tc.tile_pool(name="c", bufs=1) as cp, \
             tc.tile_pool(name="ps", bufs=2, space="PSUM") as psp, \
             tc.tile_pool(name="mlp", bufs=1) as mlp:
            pal = cp.tile([P, NC, N_GRAPHS], mybir.dt.float32, tag="pal")
            nc.sync.dma_start(
                out=pal[:], in_=pall[:].rearrange("(c p) g -> p c g", p=P))
            ic_sb = cp.tile([P, N_GRAPHS], mybir.dt.float32, tag="ic")
            nc.sync.dma_start(out=ic_sb[:], in_=invc[:])
            w0 = cp.tile([128, 256], mybir.dt.float32, tag="w0")
            nc.sync.dma_start(out=w0[:], in_=Wf0[:])
            w1 = cp.tile([128, 2, 256], mybir.dt.float32, tag="w1")
            nc.sync.dma_start(out=w1[:], in_=Wf1[:].rearrange("(b k) o -> k b o", b=2))
            wo = cp.tile([128, 2, 8], mybir.dt.float32, tag="wo")
            nc.sync.dma_start(out=wo[:], in_=Wout[:].rearrange("(b k) o -> k b o", b=2))
            b0 = cp.tile([128, 2], mybir.dt.float32, tag="b0")
            nc.sync.dma_start(out=b0[:], in_=bf0[:].rearrange("(b k) o -> k (b o)", b=2))
            b1 = cp.tile([128, 2], mybir.dt.float32, tag="b1")
            nc.sync.dma_start(out=b1[:], in_=bf1[:].rearrange("(b k) o -> k (b o)", b=2))
            bo = cp.tile([8, 1], mybir.dt.float32, tag="bo")
            nc.sync.dma_start(out=bo[:], in_=bout[:])

            acc = mlp.tile([P, N_GRAPHS], mybir.dt.float32, tag="acc")
            nc.vector.tensor_copy(out=acc[:], in_=pal[:, 0, :])
            for c in range(1, NC):
                nc.vector.tensor_add(out=acc[:], in0=acc[:], in1=pal[:, c, :])
            hgT = mlp.tile([P, N_GRAPHS], mybir.dt.float32, tag="hgT")
            nc.vector.tensor_tensor(out=hgT[:], in0=acc[:], in1=ic_sb[:],
                                    op=mybir.AluOpType.mult)

            a1_0 = mlp.tile([P, N_GRAPHS], mybir.dt.float32, tag="a1_0")
            a1_1 = mlp.tile([P, N_GRAPHS], mybir.dt.float32, tag="a1_1")
            a1 = [a1_0, a1_1]
            for ob in range(2):
                ps = psp.tile([P, N_GRAPHS], mybir.dt.float32, tag="mps")
                nc.tensor.matmul(out=ps[:], lhsT=w0[:, ob * 128:(ob + 1) * 128],
                                 rhs=hgT[:], start=True, stop=True)
                nc.vector.tensor_scalar_add(
                    out=a1[ob][:], in0=ps[:], scalar1=b0[:, ob:ob + 1])
                nc.scalar.activation(out=a1[ob][:], in_=a1[ob][:],
                                     func=mybir.ActivationFunctionType.Relu)
            a2_0 = mlp.tile([P, N_GRAPHS], mybir.dt.float32, tag="a2_0")
            a2_1 = mlp.tile([P, N_GRAPHS], mybir.dt.float32, tag="a2_1")
            a2 = [a2_0, a2_1]
            for ob in range(2):
                ps = psp.tile([P, N_GRAPHS], mybir.dt.float32, tag="mps")
                for ib in range(2):
                    nc.tensor.matmul(out=ps[:],
                                     lhsT=w1[:, ib, ob * 128:(ob + 1) * 128],
                                     rhs=a1[ib][:],
                                     start=(ib == 0), stop=(ib == 1))
                nc.vector.tensor_scalar_add(
                    out=a2[ob][:], in0=ps[:], scalar1=b1[:, ob:ob + 1])
                nc.scalar.activation(out=a2[ob][:], in_=a2[ob][:],
                                     func=mybir.ActivationFunctionType.Relu)
            ps = psp.tile([8, N_GRAPHS], mybir.dt.float32, tag="ops")
            for ib in range(2):
                nc.tensor.matmul(out=ps[:], lhsT=wo[:, ib, :], rhs=a2[ib][:],
                                 start=(ib == 0), stop=(ib == 1))
            oT = mlp.tile([8, N_GRAPHS], mybir.dt.float32, tag="oT")
            nc.vector.tensor_scalar_add(out=oT[:], in0=ps[:], scalar1=bo[:])
            nc.sync.dma_start(out=outT[:], in_=oT[:])
    nc.compile()
    return nc


def kernel(x, edge_src, edge_dst, node2graph,
           Wg0, bg0, Wg1, bg1, Wg2, bg2,
           Wf0, bf0, Wf1, bf1, Wout, bout):
    global LAST_EXEC_NS
    LAST_EXEC_NS = []
    per_core, meta, inv_cnt = _prep(edge_src, edge_dst, node2graph)

    trace = os.environ.get("GNN_TRACE", "0") == "1"

    def run(nc, in_maps):
        res = run_bass_kernel_spmd(nc, in_maps, core_ids=list(range(NC)),
                                   trace=trace)
        if res.exec_time_ns:
            LAST_EXEC_NS.append(res.exec_time_ns)
        return res

    iota128 = np.tile(np.arange(P, dtype=np.float32), (P, 1)).astype(
        ml_dtypes.bfloat16)
    iotaB128 = (np.tile(np.arange(P, dtype=np.float32), (P, 1)) + 128).astype(
        ml_dtypes.bfloat16)

    conv12 = _build_conv(meta, with_pool=False)
    conv3 = _build_conv(meta, with_pool=True)
    tailk = _build_tail()

    hT_pad = np.zeros((P, PADN), ml_dtypes.bfloat16)
    hT_pad[:, :N_NODES] = np.asarray(x, np.float32).T.astype(ml_dtypes.bfloat16)
    for li, (Wl, bl) in enumerate(((Wg0, bg0), (Wg1, bg1), (Wg2, bg2))):
        conv = conv3 if li == 2 else conv12
        in_maps = []
        for c in range(NC):
            im = dict(
                hT=hT_pad, W=np.asarray(Wl, np.float32).astype(ml_dtypes.bfloat16),
                brep=np.tile(np.asarray(bl, np.float32), (P, 1)),
                ndstc=per_core[c]["ndstc"], dl=per_core[c]["dl"],
                nsl=per_core[c]["nsl"], idx16=per_core[c]["idx16"],
                iota=iota128)
            if li == 2:
                im["gidc"] = per_core[c]["gidc"]
                im["iotaB"] = iotaB128
            in_maps.append(im)
        res = run(conv, in_maps)
        if li < 2:
            hT_pad = np.zeros((P, PADN), ml_dtypes.bfloat16)
            for c in range(NC):
                lo, hi = c * OWN, min((c + 1) * OWN, N_NODES)
                hT_pad[:, lo:hi] = res.results[c]["hout"][:hi - lo].T

    pall = np.concatenate([res.results[c]["poolT"] for c in range(NC)], axis=0)
    im = dict(pall=pall,
              invc=np.tile(inv_cnt, (P, 1)),
              Wf0=np.asarray(Wf0, np.float32),
              bf0=np.asarray(bf0, np.float32).reshape(256, 1),
              Wf1=np.asarray(Wf1, np.float32),
              bf1=np.asarray(bf1, np.float32).reshape(256, 1),
              Wout=np.asarray(Wout, np.float32),
              bout=np.asarray(bout, np.float32).reshape(8, 1))
    res = run(tailk, [dict(im) for _ in range(NC)])
    return np.ascontiguousarray(res.results[0]["outT"].T)


# revision 7
# speedup vs baseline: 1.3619x; 1.3619x over previous
"""GNN message-passing kernel for 8 Trainium2 NeuronCores.

Strategy: dst-partition nodes 8 ways (12544/core incl pad). Per GraphConv
layer (one SPMD launch; one NEFF per layer):
  A) each core computes z = h @ W for ALL nodes (replicated, bf16 PE work),
     stores z bf16 in 4 per-window HBM tensors (windows keep gather idxs
     in int16 range).
  B) per-edge messages gathered via Q7 dma_gather (int16 idxs), edges
     pre-sorted into static per-(window, dst-tile) slot groups (max over
     cores, shared NEFF schedule). Gather calls round-robin across the
     4 windows / swdge queues.
  C) segmented reduction on the PE: per 128-edge chunk a one-hot S matrix
     maps edge slots to the 128 dsts of the chunk's tile; S matrices are
     prebuilt on DVE into 64-chunk batch buffers (coarse semaphores), in
     consumption ("job") order. Layer 1 fuses nsrc[src] into S via a
     two-scalar tensor_scalar; later layers receive h pre-scaled by nsrc
     (folded into the previous layer's relu) so S is a plain one-hot
     built 8 chunks per DVE op. PSUM accumulates all chunks of a dst tile.
  D) h' = relu((psum * ndst + b) * nsrc_next): one DVE scalar_tensor_tensor
     + one ACT relu (with per-partition nsrc scale) per tile; layer-3 NEFF
     instead accumulates per-graph pool partials poolT[f,g] += hn^T @
     onehot(gid) on two persistent PSUM banks.
A tiny 4th launch sums the 8 cores' pool partials, applies 1/count and the
MLP tail (replicated on all cores). All float math on x runs on device; the
host only computes integer edge/group structure and degree norms
(graph-structure metadata) and reshapes/casts activations between launches.
"""
import sys, types, os
sys.path.insert(0, "/opt/trn_rl_repo")

try:
    import antenv.axon_hooks  # noqa: F401
except Exception:
    try:
        import antenv
        from trn_agent_boot.trn_boot import _ntff_profile_via_ctypes
        _hook = _ntff_profile_via_ctypes("/opt/axon/libaxon_pjrt.so")
        _m = types.ModuleType("antenv.axon_hooks")
        _m.get_axon_ntff_profile_hook = lambda: _hook
        _m.set_axon_ntff_profile_hook = lambda h: None
        sys.modules["antenv.axon_hooks"] = _m
        antenv.axon_hooks = _m
    except Exception:
        pass

import numpy as np
import ml_dtypes
import concourse.bacc as bacc
import concourse.mybir as mybir
import concourse.tile as tile
from concourse.bass_utils import run_bass_kernel_spmd

P = 128
N_NODES, N_EDGES, N_GRAPHS = 100000, 1600000, 256
D = 128
NC = 8
OWN = 12544                    # dst nodes per core (incl pad on core 7)
NT = OWN // P                  # 98 dst tiles per core
NW = 4                         # z windows (int16 gather idx range)
WIN = 25088                    # rows per window
PADN = NW * WIN                # 100352 padded node rows
GCALL = 4096                   # max edges per dma_gather call
SBATCH = 64                    # S matrices per prebuilt batch buffer
NOMATCH = 240.0                # dl value that never matches iota 0..127

LAST_EXEC_NS = []


def _pack_idxs(idx):
    n = len(idx)
    S = (n + 15) // 16
    arr = np.zeros((16, S), dtype=np.int16)
    arr[np.arange(n) % 16, np.arange(n) // 16] = idx.astype(np.int16)
    return np.tile(arr, (8, 1))


def _prep(edge_src, edge_dst, node2graph):
    es = np.asarray(edge_src).astype(np.int64)
    ed = np.asarray(edge_dst).astype(np.int64)
    n2g = np.asarray(node2graph)
    out_deg = np.bincount(es, minlength=N_NODES).astype(np.float32)
    in_deg = np.bincount(ed, minlength=N_NODES).astype(np.float32)
    nsrc = 1.0 / np.sqrt(np.maximum(out_deg, 1.0))
    ndst = 1.0 / np.sqrt(np.maximum(in_deg, 1.0))

    NG = NW * NT  # 392 groups
    cnts = np.zeros((NC, NG), np.int64)
    core_data = []
    for c in range(NC):
        m = (ed // OWN) == c
        s, d = es[m], ed[m]
        dl = d - OWN * c
        t = dl >> 7
        w = s // WIN
        key = w * NT + t
        order = np.argsort(key, kind="stable")
        cnts[c] = np.bincount(key, minlength=NG)
        core_data.append((s[order], dl[order], key[order]))

    slots_g = (((cnts.max(axis=0) + P - 1) // P) * P).astype(np.int64)  # [392]
    chunks_g = slots_g // P
    group_start = np.zeros(NG + 1, np.int64)
    group_start[1:] = np.cumsum(slots_g)
    tot_slots = int(group_start[-1])
    tot_chunks = tot_slots // P

    # per-stream static call plan (streams are contiguous group ranges)
    stream_slots = [int(slots_g[w * NT:(w + 1) * NT].sum()) for w in range(NW)]
    stream_base = np.zeros(NW + 1, np.int64)
    stream_base[1:] = np.cumsum(stream_slots)
    calls = []           # [(w, n_slots), ...] in round-robin issue order
    calls_per_w = []
    for w in range(NW):
        r, lst = stream_slots[w], []
        while r > 0:
            n = min(GCALL, r)
            lst.append(n)
            r -= n
        calls_per_w.append(lst)
    ncall_max = max(len(l) for l in calls_per_w)
    for ci in range(ncall_max):
        for w in range(NW):
            if ci < len(calls_per_w[w]):
                calls.append((w, calls_per_w[w][ci]))

    # chunk -> (stream call index, slot in call) static map
    chunk_map = {}
    for w in range(NW):
        off = 0
        for ci, n in enumerate(calls_per_w[w]):
            for k in range(n // P):
                chunk_map[(w, off // P + k)] = (ci, k)
            off += n

    # job schedule: tile-major over (stream, chunk); jobs[ji] = (w, cw, gc)
    gchunk_base = np.zeros(NW, np.int64)
    off = 0
    for w in range(NW):
        gchunk_base[w] = off
        off += stream_slots[w] // P
    group_chunk_start = {}
    for w in range(NW):
        o = 0
        for t in range(NT):
            group_chunk_start[(w, t)] = o
            o += int(chunks_g[w * NT + t])
    jobs = []            # global job order
    tile_njobs = []
    for t in range(NT):
        nj = 0
        for w in range(NW):
            for k in range(int(chunks_g[w * NT + t])):
                cw = group_chunk_start[(w, t)] + k
                jobs.append((w, cw, int(gchunk_base[w]) + cw))
                nj += 1
        tile_njobs.append(nj)
    njobs = len(jobs)
    njobs_pad = ((njobs + SBATCH - 1) // SBATCH) * SBATCH
    job_gc = np.array([j[2] for j in jobs], np.int64)   # job -> global chunk

    per_core = []
    for c in range(NC):
        s, dl, key = core_data[c]
        idx_stream = np.zeros(tot_slots, np.int64)
        dl_stream = np.full(tot_slots, NOMATCH, np.float64)
        ns_stream = np.zeros(tot_slots, np.float64)
        if len(key):
            gidx = group_start[key] + np.concatenate(
                [np.arange(n) for n in np.bincount(key, minlength=NG)])
            idx_stream[gidx] = s % WIN
            dl_stream[gidx] = dl & 127
            ns_stream[gidx] = nsrc[s]
        packs = [_pack_idxs(idx_stream[stream_base[w]:stream_base[w + 1]])
                 for w in range(NW)]
        idx16 = np.concatenate(packs, axis=1)
        dlc_chunk = dl_stream.reshape(tot_chunks, P).T.astype(np.float32)
        nsl_chunk = ns_stream.reshape(tot_chunks, P).T.astype(np.float32)
        # permute columns to JOB order, pad to njobs_pad
        dlc = np.full((P, njobs_pad), NOMATCH, np.float32)
        dlc[:, :njobs] = dlc_chunk[:, job_gc]
        nsl = np.zeros((P, njobs_pad), np.float32)
        nsl[:, :njobs] = nsl_chunk[:, job_gc]

        gid = np.full(OWN, -1.0, np.float32)
        lo, hi = c * OWN, min((c + 1) * OWN, N_NODES)
        gid[:hi - lo] = n2g[lo:hi]
        nd = np.zeros(OWN, np.float32)
        nd[:hi - lo] = ndst[lo:hi]
        ns_own = np.zeros(OWN, np.float32)
        ns_own[:hi - lo] = nsrc[lo:hi]
        per_core.append(dict(
            idx16=idx16, dl=dlc, nsl=nsl,
            ndstc=nd.reshape(NT, P).T.copy(),
            nsrcc=ns_own.reshape(NT, P).T.copy(),
            gidc=gid.reshape(NT, P).T.copy()))

    cnt = np.bincount(n2g, minlength=N_GRAPHS).astype(np.float32)
    inv_cnt = (1.0 / np.maximum(cnt, 1.0))

    meta = dict(slots_g=slots_g, chunks_g=chunks_g, tot_slots=tot_slots,
                tot_chunks=tot_chunks, calls=calls, calls_per_w=calls_per_w,
                chunk_map=chunk_map, jobs=jobs, tile_njobs=tile_njobs,
                njobs=njobs, njobs_pad=njobs_pad)
    return per_core, meta, inv_cnt


def _stream_slot_off(meta, w):
    off = 0
    for ww in range(w):
        off += sum(meta["calls_per_w"][ww])
    return off


def _build_conv(meta, fuse_nsl, fold_nsrc, with_pool):
    calls = meta["calls"]
    calls_per_w = meta["calls_per_w"]
    chunk_map = meta["chunk_map"]
    jobs = meta["jobs"]
    tile_njobs = meta["tile_njobs"]
    njobs_pad = meta["njobs_pad"]
    tot_slots = meta["tot_slots"]
    IDXC = tot_slots // 16
    K8 = 8

    nc = bacc.Bacc("TRN2", num_devices=NC, num_swdge_queues=4)
    hT = nc.dram_tensor("hT", [P, PADN], mybir.dt.bfloat16, kind="ExternalInput")
    W = nc.dram_tensor("W", [D, D], mybir.dt.bfloat16, kind="ExternalInput")
    brep = nc.dram_tensor("brep", [P, D], mybir.dt.float32, kind="ExternalInput")
    ndstc = nc.dram_tensor("ndstc", [P, NT], mybir.dt.float32, kind="ExternalInput")
    dl = nc.dram_tensor("dl", [P, njobs_pad], mybir.dt.float32,
                        kind="ExternalInput")
    if fuse_nsl:
        nsl = nc.dram_tensor("nsl", [P, njobs_pad], mybir.dt.float32,
                             kind="ExternalInput")
    if fold_nsrc:
        nsrcc = nc.dram_tensor("nsrcc", [P, NT], mybir.dt.float32,
                               kind="ExternalInput")
    idx16 = nc.dram_tensor("idx16", [P, IDXC], mybir.dt.int16, kind="ExternalInput")
    iota = nc.dram_tensor("iota", [P, P], mybir.dt.bfloat16, kind="ExternalInput")
    iota8 = nc.dram_tensor("iota8", [P, K8, P], mybir.dt.bfloat16,
                           kind="ExternalInput")
    if with_pool:
        gidc = nc.dram_tensor("gidc", [P, NT], mybir.dt.float32,
                              kind="ExternalInput")
        iotaB = nc.dram_tensor("iotaB", [P, P], mybir.dt.bfloat16,
                               kind="ExternalInput")
        poolT = nc.dram_tensor("poolT", [P, N_GRAPHS], mybir.dt.float32,
                               kind="ExternalOutput")
    else:
        hout = nc.dram_tensor("hout", [OWN, D], mybir.dt.bfloat16,
                              kind="ExternalOutput")
    zw = [nc.dram_tensor(f"z{w}", [WIN, D], mybir.dt.bfloat16) for w in range(NW)]

    with tile.TileContext(nc) as tc:
        with tc.tile_pool(name="const", bufs=1) as cp, \
             tc.tile_pool(name="hblk", bufs=2) as hp, \
             tc.tile_pool(name="zst", bufs=4) as zp, \
             tc.tile_pool(name="zps", bufs=2, space="PSUM") as zps, \
             tc.tile_pool(name="m0", bufs=2) as mp0, \
             tc.tile_pool(name="m1", bufs=2) as mp1, \
             tc.tile_pool(name="m2", bufs=2) as mp2, \
             tc.tile_pool(name="m3", bufs=2) as mp3, \
             tc.tile_pool(name="sbat", bufs=2) as sbp, \
             tc.tile_pool(name="sg", bufs=4) as sgp, \
             tc.tile_pool(name="cps", bufs=3, space="PSUM") as cpsp, \
             tc.tile_pool(name="pps", bufs=1, space="PSUM") as ppsp, \
             tc.tile_pool(name="dph", bufs=3) as dp:
            mpools = [mp0, mp1, mp2, mp3]
            W_sb = cp.tile([D, D], mybir.dt.bfloat16, tag="W")
            nc.sync.dma_start(out=W_sb[:], in_=W[:])
            brep_sb = cp.tile([P, D], mybir.dt.float32, tag="brep")
            nc.sync.dma_start(out=brep_sb[:], in_=brep[:])
            ndst_sb = cp.tile([P, NT], mybir.dt.float32, tag="ndst")
            nc.sync.dma_start(out=ndst_sb[:], in_=ndstc[:])
            dl_sb = cp.tile([P, njobs_pad], mybir.dt.float32, tag="dl")
            nc.sync.dma_start(out=dl_sb[:], in_=dl[:])
            if fuse_nsl:
                nsl_sb = cp.tile([P, njobs_pad], mybir.dt.float32, tag="nsl")
                nc.sync.dma_start(out=nsl_sb[:], in_=nsl[:])
            if fold_nsrc:
                nsrc_sb = cp.tile([P, NT], mybir.dt.float32, tag="nsrc")
                nc.sync.dma_start(out=nsrc_sb[:], in_=nsrcc[:])
            idx_sb = cp.tile([P, IDXC], mybir.dt.int16, tag="idx")
            nc.sync.dma_start(out=idx_sb[:], in_=idx16[:])
            iota_sb = cp.tile([P, P], mybir.dt.bfloat16, tag="iota")
            nc.sync.dma_start(out=iota_sb[:], in_=iota[:])
            iota8_sb = cp.tile([P, K8, P], mybir.dt.bfloat16, tag="iota8")
            nc.sync.dma_start(out=iota8_sb[:], in_=iota8[:])
            if with_pool:
                gid_sb = cp.tile([P, NT], mybir.dt.float32, tag="gid")
                nc.sync.dma_start(out=gid_sb[:], in_=gidc[:])
                iotaB_sb = cp.tile([P, P], mybir.dt.bfloat16, tag="iotaB")
                nc.sync.dma_start(out=iotaB_sb[:], in_=iotaB[:])
                pool_ps0 = ppsp.tile([P, P], mybir.dt.float32, tag="pool0")
                pool_ps1 = ppsp.tile([P, P], mybir.dt.float32, tag="pool1")
                pool_ps = [pool_ps0, pool_ps1]

            # ---- phase A: z = h @ W  (bf16), per window ----
            ZB = 4   # chunks per psum bank / staging store
            HB = 28  # chunks per hT staging block
            WCH = WIN // P  # 196 chunks per window
            for w in range(NW):
                zv = zw[w][:].rearrange("(a k n) f -> a n k f", n=P, k=ZB)
                for blk in range(WCH // HB):
                    c0 = blk * HB
                    hT_sb = hp.tile([P, HB * P], mybir.dt.bfloat16, tag="h")
                    nc.sync.dma_start(
                        out=hT_sb[:],
                        in_=hT[:, w * WIN + c0 * P:w * WIN + (c0 + HB) * P])
                    for g0 in range(0, HB, ZB):
                        ps = zps.tile([P, ZB, D], mybir.dt.float32, tag="zps")
                        for k in range(ZB):
                            nc.tensor.matmul(
                                out=ps[:, k, :],
                                lhsT=hT_sb[:, (g0 + k) * P:(g0 + k + 1) * P],
                                rhs=W_sb[:], start=True, stop=True)
                        zst = zp.tile([P, ZB, D], mybir.dt.bfloat16, tag="zst")
                        nc.scalar.activation(
                            out=zst[:], in_=ps[:],
                            func=mybir.ActivationFunctionType.Copy)
                        nc.sync.dma_start(out=zv[(c0 + g0) // ZB], in_=zst[:])

            # ---- phase B: gathers, round-robin across windows/queues ----
            msg_tiles = {w: [] for w in range(NW)}
            woff = [0] * NW
            for (w, n) in calls:
                mt = mpools[w].tile([P, n // P, D], mybir.dt.bfloat16,
                                    tag=f"msg{w}")
                so = _stream_slot_off(meta, w) + woff[w]
                nc.gpsimd.dma_gather(
                    mt[:], zw[w][:], idx_sb[:, so // 16:(so + n) // 16],
                    n, n, D, queue_num=w, single_packet=False)
                msg_tiles[w].append(mt)
                woff[w] += n

            # ---- phase C+D: tile-major segmented reduce with prebuilt S ----
            sbats = []

            def ensure_sbat(ji):
                b = ji // SBATCH
                while len(sbats) <= b:
                    j0 = len(sbats) * SBATCH
                    SB = sbp.tile([P, SBATCH, P], mybir.dt.bfloat16, tag="SB")
                    if fuse_nsl:
                        for k in range(SBATCH):
                            nc.vector.tensor_scalar(
                                out=SB[:, k, :], in0=iota_sb[:],
                                scalar1=dl_sb[:, j0 + k:j0 + k + 1],
                                scalar2=nsl_sb[:, j0 + k:j0 + k + 1],
                                op0=mybir.AluOpType.is_equal,
                                op1=mybir.AluOpType.mult)
                    else:
                        for k in range(0, SBATCH, K8):
                            nc.vector.tensor_tensor(
                                out=SB[:, k:k + K8, :],
                                in0=dl_sb[:, j0 + k:j0 + k + K8]
                                .to_broadcast([P, K8, P]),
                                in1=iota8_sb[:],
                                op=mybir.AluOpType.is_equal)
                    sbats.append(SB)
                return sbats[b]

            ji = 0
            for t in range(NT):
                nj = tile_njobs[t]
                ps = cpsp.tile([P, D], mybir.dt.float32, tag="cps")
                for u in range(nj):
                    w, cw, gc = jobs[ji]
                    SB = ensure_sbat(ji)
                    ci, slot = chunk_map[(w, cw)]
                    nc.tensor.matmul(
                        out=ps[:], lhsT=SB[:, ji % SBATCH, :],
                        rhs=msg_tiles[w][ci][:, slot, :],
                        start=(u == 0), stop=(u == nj - 1))
                    ji += 1
                hn = dp.tile([P, D], mybir.dt.bfloat16, tag="hn")
                nc.vector.scalar_tensor_tensor(
                    out=hn[:], in0=ps[:], scalar=ndst_sb[:, t:t + 1],
                    in1=brep_sb[:], op0=mybir.AluOpType.mult,
                    op1=mybir.AluOpType.add)
                if fold_nsrc:
                    nc.scalar.activation(
                        out=hn[:], in_=hn[:],
                        func=mybir.ActivationFunctionType.Relu,
                        scale=nsrc_sb[:, t:t + 1])
                else:
                    nc.scalar.activation(
                        out=hn[:], in_=hn[:],
                        func=mybir.ActivationFunctionType.Relu)
                if with_pool:
                    for b, io_sb in enumerate((iota_sb, iotaB_sb)):
                        Sg = sgp.tile([P, P], mybir.dt.bfloat16, tag="Sg")
                        nc.vector.tensor_single_scalar(
                            out=Sg[:], in_=io_sb[:],
                            scalar=gid_sb[:, t:t + 1],
                            op=mybir.AluOpType.is_equal)
                        nc.tensor.matmul(
                            out=pool_ps[b][:], lhsT=hn[:], rhs=Sg[:],
                            start=(t == 0), stop=(t == NT - 1))
                else:
                    nc.sync.dma_start(out=hout[t * P:(t + 1) * P, :], in_=hn[:])

            if with_pool:
                pool_sb = dp.tile([P, 2, P], mybir.dt.float32, tag="poolsb")
                for b in range(2):
                    nc.vector.tensor_copy(out=pool_sb[:, b, :], in_=pool_ps[b][:])
                nc.sync.dma_start(
                    out=poolT[:], in_=pool_sb[:].rearrange("p b g -> p (b g)"))
    nc.compile()
    return nc


def _build_tail():
    nc = bacc.Bacc("TRN2", num_devices=NC, num_swdge_queues=1)
    pall = nc.dram_tensor("pall", [NC * P, N_GRAPHS], mybir.dt.float32,
                          kind="ExternalInput")
    invc = nc.dram_tensor("invc", [P, N_GRAPHS], mybir.dt.float32,
                          kind="ExternalInput")
    Wf0 = nc.dram_tensor("Wf0", [128, 256], mybir.dt.float32, kind="ExternalInput")
    bf0 = nc.dram_tensor("bf0", [256, 1], mybir.dt.float32, kind="ExternalInput")
    Wf1 = nc.dram_tensor("Wf1", [256, 256], mybir.dt.float32, kind="ExternalInput")
    bf1 = nc.dram_tensor("bf1", [256, 1], mybir.dt.float32, kind="ExternalInput")
    Wout = nc.dram_tensor("Wout", [256, 8], mybir.dt.float32, kind="ExternalInput")
    bout = nc.dram_tensor("bout", [8, 1], mybir.dt.float32, kind="ExternalInput")
    outT = nc.dram_tensor("outT", [8, N_GRAPHS], mybir.dt.float32,
                          kind="ExternalOutput")

    with tile.TileContext(nc) as tc:
        with tc.tile_pool(name="c", bufs=1) as cp, \
             tc.tile_pool(name="ps", bufs=2, space="PSUM") as psp, \
             tc.tile_pool(name="mlp", bufs=1) as mlp:
            pal = cp.tile([P, NC, N_GRAPHS], mybir.dt.float32, tag="pal")
            nc.sync.dma_start(
                out=pal[:], in_=pall[:].rearrange("(c p) g -> p c g", p=P))
            ic_sb = cp.tile([P, N_GRAPHS], mybir.dt.float32, tag="ic")
            nc.sync.dma_start(out=ic_sb[:], in_=invc[:])
            w0 = cp.tile([128, 256], mybir.dt.float32, tag="w0")
            nc.sync.dma_start(out=w0[:], in_=Wf0[:])
            w1 = cp.tile([128, 2, 256], mybir.dt.float32, tag="w1")
            nc.sync.dma_start(out=w1[:], in_=Wf1[:].rearrange("(b k) o -> k b o", b=2))
            wo = cp.tile([128, 2, 8], mybir.dt.float32, tag="wo")
            nc.sync.dma_start(out=wo[:], in_=Wout[:].rearrange("(b k) o -> k b o", b=2))
            b0 = cp.tile([128, 2], mybir.dt.float32, tag="b0")
            nc.sync.dma_start(out=b0[:], in_=bf0[:].rearrange("(b k) o -> k (b o)", b=2))
            b1 = cp.tile([128, 2], mybir.dt.float32, tag="b1")
            nc.sync.dma_start(out=b1[:], in_=bf1[:].rearrange("(b k) o -> k (b o)", b=2))
            bo = cp.tile([8, 1], mybir.dt.float32, tag="bo")
            nc.sync.dma_start(out=bo[:], in_=bout[:])

            acc = mlp.tile([P, N_GRAPHS], mybir.dt.float32, tag="acc")
            nc.vector.tensor_copy(out=acc[:], in_=pal[:, 0, :])
            for c in range(1, NC):
                nc.vector.tensor_add(out=acc[:], in0=acc[:], in1=pal[:, c, :])
            hgT = mlp.tile([P, N_GRAPHS], mybir.dt.float32, tag="hgT")
            nc.vector.tensor_tensor(out=hgT[:], in0=acc[:], in1=ic_sb[:],
                                    op=mybir.AluOpType.mult)

            a1_0 = mlp.tile([P, N_GRAPHS], mybir.dt.float32, tag="a1_0")
            a1_1 = mlp.tile([P, N_GRAPHS], mybir.dt.float32, tag="a1_1")
            a1 = [a1_0, a1_1]
            for ob in range(2):
                ps = psp.tile([P, N_GRAPHS], mybir.dt.float32, tag="mps")
                nc.tensor.matmul(out=ps[:], lhsT=w0[:, ob * 128:(ob + 1) * 128],
                                 rhs=hgT[:], start=True, stop=True)
                nc.vector.tensor_scalar_add(
                    out=a1[ob][:], in0=ps[:], scalar1=b0[:, ob:ob + 1])
                nc.scalar.activation(out=a1[ob][:], in_=a1[ob][:],
                                     func=mybir.ActivationFunctionType.Relu)
            a2_0 = mlp.tile([P, N_GRAPHS], mybir.dt.float32, tag="a2_0")
            a2_1 = mlp.tile([P, N_GRAPHS], mybir.dt.float32, tag="a2_1")
            a2 = [a2_0, a2_1]
            for ob in range(2):
                ps = psp.tile([P, N_GRAPHS], mybir.dt.float32, tag="mps")
                for ib in range(2):
                    nc.tensor.matmul(out=ps[:],
                                     lhsT=w1[:, ib, ob * 128:(ob + 1) * 128],
                                     rhs=a1[ib][:],
                                     start=(ib == 0), stop=(ib == 1))
                nc.vector.tensor_scalar_add(
                    out=a2[ob][:], in0=ps[:], scalar1=b1[:, ob:ob + 1])
                nc.scalar.activation(out=a2[ob][:], in_=a2[ob][:],
                                     func=mybir.ActivationFunctionType.Relu)
            ps = psp.tile([8, N_GRAPHS], mybir.dt.float32, tag="ops")
            for ib in range(2):
                nc.tensor.matmul(out=ps[:], lhsT=wo[:, ib, :], rhs=a2[ib][:],
                                 start=(ib == 0), stop=(ib == 1))
            oT = mlp.tile([8, N_GRAPHS], mybir.dt.float32, tag="oT")
            nc.vector.tensor_scalar_add(out=oT[:], in0=ps[:], scalar1=bo[:])
            nc.sync.dma_start(out=outT[:], in_=oT[:])
    nc.compile()
    return nc


def kernel(x, edge_src, edge_dst, node2graph,
           Wg0, bg0, Wg1, bg1, Wg2, bg2,
           Wf0, bf0, Wf1, bf1, Wout, bout):
    global LAST_EXEC_NS
    LAST_EXEC_NS = []
    per_core, meta, inv_cnt = _prep(edge_src, edge_dst, node2graph)

    trace = os.environ.get("GNN_TRACE", "0") == "1"

    def run(nc, in_maps):
        res = run_bass_kernel_spmd(nc, in_maps, core_ids=list(range(NC)),
                                   trace=trace)
        if res.exec_time_ns:
            LAST_EXEC_NS.append(res.exec_time_ns)
        return res

    iota128 = np.tile(np.arange(P, dtype=np.float32), (P, 1)).astype(
        ml_dtypes.bfloat16)
    iotaB128 = (np.tile(np.arange(P, dtype=np.float32), (P, 1)) + 128).astype(
        ml_dtypes.bfloat16)
    iota8 = np.tile(np.arange(P, dtype=np.float32), (P, 8)).astype(
        ml_dtypes.bfloat16).reshape(P, 8, P)

    conv1 = _build_conv(meta, fuse_nsl=True, fold_nsrc=True, with_pool=False)
    conv2 = _build_conv(meta, fuse_nsl=False, fold_nsrc=True, with_pool=False)
    conv3 = _build_conv(meta, fuse_nsl=False, fold_nsrc=False, with_pool=True)
    tailk = _build_tail()

    hT_pad = np.zeros((P, PADN), ml_dtypes.bfloat16)
    hT_pad[:, :N_NODES] = np.asarray(x, np.float32).T.astype(ml_dtypes.bfloat16)
    for li, (conv, Wl, bl) in enumerate(((conv1, Wg0, bg0), (conv2, Wg1, bg1),
                                         (conv3, Wg2, bg2))):
        in_maps = []
        for c in range(NC):
            im = dict(
                hT=hT_pad, W=np.asarray(Wl, np.float32).astype(ml_dtypes.bfloat16),
                brep=np.tile(np.asarray(bl, np.float32), (P, 1)),
                ndstc=per_core[c]["ndstc"], dl=per_core[c]["dl"],
                idx16=per_core[c]["idx16"],
                iota=iota128, iota8=iota8)
            if li == 0:
                im["nsl"] = per_core[c]["nsl"]
            if li < 2:
                im["nsrcc"] = per_core[c]["nsrcc"]
            else:
                im["gidc"] = per_core[c]["gidc"]
                im["iotaB"] = iotaB128
            in_maps.append(im)
        res = run(conv, in_maps)
        if li < 2:
            hT_pad = np.zeros((P, PADN), ml_dtypes.bfloat16)
            for c in range(NC):
                lo, hi = c * OWN, min((c + 1) * OWN, N_NODES)
                hT_pad[:, lo:hi] = res.results[c]["hout"][:hi - lo].T

    pall = np.concatenate([res.results[c]["poolT"] for c in range(NC)], axis=0)
    im = dict(pall=pall,
              invc=np.tile(inv_cnt, (P, 1)),
              Wf0=np.asarray(Wf0, np.float32),
              bf0=np.asarray(bf0, np.float32).reshape(256, 1),
              Wf1=np.asarray(Wf1, np.float32),
              bf1=np.asarray(bf1, np.float32).reshape(256, 1),
              Wout=np.asarray(Wout, np.float32),
              bout=np.asarray(bout, np.float32).reshape(8, 1))
    res = run(tailk, [dict(im) for _ in range(NC)])
    return np.ascontiguousarray(res.results[0]["outT"].T)


# revision 12
# speedup vs baseline: 1.4696x; 1.0791x over previous
"""GNN message-passing kernel for 8 Trainium2 NeuronCores.

Strategy: dst-partition nodes 8 ways (12544/core incl pad). Per GraphConv
layer (one SPMD launch; one NEFF per layer):
  A) each core computes z = h @ W for ALL nodes (replicated, bf16 PE work),
     stores z bf16 in 4 per-window HBM tensors (windows keep gather idxs
     in int16 range).
  B) per-edge messages gathered via Q7 dma_gather (int16 idxs), edges
     pre-sorted into static per-(window, dst-tile) slot groups (max over
     cores, shared NEFF schedule). Gather calls round-robin across the
     4 windows / swdge queues.
  C) segmented reduction on the PE: per 128-edge chunk a one-hot S matrix
     maps edge slots to the 128 dsts of the chunk's tile; S matrices are
     prebuilt on DVE into 64-chunk batch buffers (coarse semaphores), in
     consumption ("job") order. Layer 1 fuses nsrc[src] into S via a
     two-scalar tensor_scalar; later layers receive h pre-scaled by nsrc
     (folded into the previous layer's relu) so S is a plain one-hot
     built 8 chunks per DVE op. PSUM accumulates all chunks of a dst tile.
  D) h' = relu((psum * ndst + b) * nsrc_next): one DVE scalar_tensor_tensor
     + one ACT relu (with per-partition nsrc scale) per tile; layer-3 NEFF
     instead accumulates per-graph pool partials poolT[f,g] += hn^T @
     onehot(gid) on two persistent PSUM banks.
A tiny 4th launch sums the 8 cores' pool partials, applies 1/count and the
MLP tail (replicated on all cores). All float math on x runs on device; the
host only computes integer edge/group structure and degree norms
(graph-structure metadata) and reshapes/casts activations between launches.
"""
import sys, types, os
sys.path.insert(0, "/opt/trn_rl_repo")

try:
    import antenv.axon_hooks  # noqa: F401
except Exception:
    try:
        import antenv
        from trn_agent_boot.trn_boot import _ntff_profile_via_ctypes
        _hook = _ntff_profile_via_ctypes("/opt/axon/libaxon_pjrt.so")
        _m = types.ModuleType("antenv.axon_hooks")
        _m.get_axon_ntff_profile_hook = lambda: _hook
        _m.set_axon_ntff_profile_hook = lambda h: None
        sys.modules["antenv.axon_hooks"] = _m
        antenv.axon_hooks = _m
    except Exception:
        pass

import numpy as np
import ml_dtypes
import concourse.bacc as bacc
import concourse.mybir as mybir
import concourse.tile as tile
from concourse.bass_utils import run_bass_kernel_spmd

P = 128
N_NODES, N_EDGES, N_GRAPHS = 100000, 1600000, 256
D = 128
NC = 8
OWN = 12544                    # dst nodes per core (incl pad on core 7)
NT = OWN // P                  # 98 dst tiles per core
NW = 4                         # z windows (int16 gather idx range)
WIN = 25088                    # rows per window
PADN = NW * WIN                # 100352 padded node rows
GCALL = 4096                   # max edges per dma_gather call
SBATCH = 64                    # S matrices per prebuilt batch buffer
NOMATCH = 240.0                # dl value that never matches iota 0..127

LAST_EXEC_NS = []


def _pack_idxs(idx):
    n = len(idx)
    S = (n + 15) // 16
    arr = np.zeros((16, S), dtype=np.int16)
    arr[np.arange(n) % 16, np.arange(n) // 16] = idx.astype(np.int16)
    return np.tile(arr, (8, 1))


def _prep(edge_src, edge_dst, node2graph):
    es = np.asarray(edge_src).astype(np.int64)
    ed = np.asarray(edge_dst).astype(np.int64)
    n2g = np.asarray(node2graph)
    out_deg = np.bincount(es, minlength=N_NODES).astype(np.float32)
    in_deg = np.bincount(ed, minlength=N_NODES).astype(np.float32)
    nsrc = 1.0 / np.sqrt(np.maximum(out_deg, 1.0))
    ndst = 1.0 / np.sqrt(np.maximum(in_deg, 1.0))

    NG = NW * NT  # 392 groups
    cnts = np.zeros((NC, NG), np.int64)
    core_data = []
    for c in range(NC):
        m = (ed // OWN) == c
        s, d = es[m], ed[m]
        dl = d - OWN * c
        t = dl >> 7
        w = s // WIN
        key = w * NT + t
        order = np.argsort(key, kind="stable")
        cnts[c] = np.bincount(key, minlength=NG)
        core_data.append((s[order], dl[order], key[order]))

    slots_g = (((cnts.max(axis=0) + P - 1) // P) * P).astype(np.int64)  # [392]
    chunks_g = slots_g // P
    group_start = np.zeros(NG + 1, np.int64)
    group_start[1:] = np.cumsum(slots_g)
    tot_slots = int(group_start[-1])
    tot_chunks = tot_slots // P

    # per-stream static call plan (streams are contiguous group ranges)
    stream_slots = [int(slots_g[w * NT:(w + 1) * NT].sum()) for w in range(NW)]
    stream_base = np.zeros(NW + 1, np.int64)
    stream_base[1:] = np.cumsum(stream_slots)
    calls = []           # [(w, n_slots), ...] in round-robin issue order
    calls_per_w = []
    for w in range(NW):
        r, lst = stream_slots[w], []
        while r > 0:
            n = min(GCALL, r)
            lst.append(n)
            r -= n
        calls_per_w.append(lst)
    ncall_max = max(len(l) for l in calls_per_w)
    for ci in range(ncall_max):
        for w in range(NW):
            if ci < len(calls_per_w[w]):
                calls.append((w, calls_per_w[w][ci]))

    # chunk -> (stream call index, slot in call) static map
    chunk_map = {}
    for w in range(NW):
        off = 0
        for ci, n in enumerate(calls_per_w[w]):
            for k in range(n // P):
                chunk_map[(w, off // P + k)] = (ci, k)
            off += n

    # job schedule: tile-major over (stream, chunk); jobs[ji] = (w, cw, gc)
    gchunk_base = np.zeros(NW, np.int64)
    off = 0
    for w in range(NW):
        gchunk_base[w] = off
        off += stream_slots[w] // P
    group_chunk_start = {}
    for w in range(NW):
        o = 0
        for t in range(NT):
            group_chunk_start[(w, t)] = o
            o += int(chunks_g[w * NT + t])
    jobs = []            # global job order
    tile_njobs = []
    for t in range(NT):
        nj = 0
        for w in range(NW):
            for k in range(int(chunks_g[w * NT + t])):
                cw = group_chunk_start[(w, t)] + k
                jobs.append((w, cw, int(gchunk_base[w]) + cw))
                nj += 1
        tile_njobs.append(nj)
    njobs = len(jobs)
    njobs_pad = ((njobs + SBATCH - 1) // SBATCH) * SBATCH
    job_gc = np.array([j[2] for j in jobs], np.int64)   # job -> global chunk

    per_core = []
    for c in range(NC):
        s, dl, key = core_data[c]
        idx_stream = np.zeros(tot_slots, np.int64)
        dl_stream = np.full(tot_slots, NOMATCH, np.float64)
        ns_stream = np.zeros(tot_slots, np.float64)
        if len(key):
            gidx = group_start[key] + np.concatenate(
                [np.arange(n) for n in np.bincount(key, minlength=NG)])
            sl = s % WIN
            # z is stored partition-major: window node n lives at row
            # (n % 128) * 196 + n // 128 (see phase A store pattern)
            idx_stream[gidx] = (sl & 127) * (WIN // P) + (sl >> 7)
            dl_stream[gidx] = dl & 127
            ns_stream[gidx] = nsrc[s]
        packs = [_pack_idxs(idx_stream[stream_base[w]:stream_base[w + 1]])
                 for w in range(NW)]
        idx16 = np.concatenate(packs, axis=1)
        dlc_chunk = dl_stream.reshape(tot_chunks, P).T.astype(np.float32)
        nsl_chunk = ns_stream.reshape(tot_chunks, P).T.astype(np.float32)
        # permute columns to JOB order, pad to njobs_pad
        dlc = np.full((P, njobs_pad), NOMATCH, np.float32)
        dlc[:, :njobs] = dlc_chunk[:, job_gc]
        nsl = np.zeros((P, njobs_pad), np.float32)
        nsl[:, :njobs] = nsl_chunk[:, job_gc]

        gid = np.full(OWN, -1.0, np.float32)
        lo, hi = c * OWN, min((c + 1) * OWN, N_NODES)
        gid[:hi - lo] = n2g[lo:hi]
        nd = np.zeros(OWN, np.float32)
        nd[:hi - lo] = ndst[lo:hi]
        ns_own = np.zeros(OWN, np.float32)
        ns_own[:hi - lo] = nsrc[lo:hi]
        per_core.append(dict(
            idx16=idx16, dl=dlc, nsl=nsl,
            ndstc=nd.reshape(NT, P).T.copy(),
            nsrcc=ns_own.reshape(NT, P).T.copy(),
            gidc=gid.reshape(NT, P).T.copy()))

    cnt = np.bincount(n2g, minlength=N_GRAPHS).astype(np.float32)
    inv_cnt = (1.0 / np.maximum(cnt, 1.0))

    meta = dict(slots_g=slots_g, chunks_g=chunks_g, tot_slots=tot_slots,
                tot_chunks=tot_chunks, calls=calls, calls_per_w=calls_per_w,
                chunk_map=chunk_map, jobs=jobs, tile_njobs=tile_njobs,
                njobs=njobs, njobs_pad=njobs_pad)
    return per_core, meta, inv_cnt


def _stream_slot_off(meta, w):
    off = 0
    for ww in range(w):
        off += sum(meta["calls_per_w"][ww])
    return off


def _build_conv(meta, fuse_nsl, fold_nsrc, with_pool):
    calls = meta["calls"]
    calls_per_w = meta["calls_per_w"]
    chunk_map = meta["chunk_map"]
    jobs = meta["jobs"]
    tile_njobs = meta["tile_njobs"]
    njobs_pad = meta["njobs_pad"]
    tot_slots = meta["tot_slots"]
    IDXC = tot_slots // 16
    K8 = 8

    nc = bacc.Bacc("TRN2", num_devices=NC, num_swdge_queues=4)
    hT = nc.dram_tensor("hT", [P, PADN], mybir.dt.bfloat16, kind="ExternalInput")
    W = nc.dram_tensor("W", [D, D], mybir.dt.bfloat16, kind="ExternalInput")
    brep = nc.dram_tensor("brep", [P, D], mybir.dt.float32, kind="ExternalInput")
    ndstc = nc.dram_tensor("ndstc", [P, NT], mybir.dt.float32, kind="ExternalInput")
    dl = nc.dram_tensor("dl", [P, njobs_pad], mybir.dt.float32,
                        kind="ExternalInput")
    if fuse_nsl:
        nsl = nc.dram_tensor("nsl", [P, njobs_pad], mybir.dt.float32,
                             kind="ExternalInput")
    if fold_nsrc:
        nsrcc = nc.dram_tensor("nsrcc", [P, NT], mybir.dt.float32,
                               kind="ExternalInput")
    idx16 = nc.dram_tensor("idx16", [P, IDXC], mybir.dt.int16, kind="ExternalInput")
    iota = nc.dram_tensor("iota", [P, P], mybir.dt.bfloat16, kind="ExternalInput")
    iota8 = nc.dram_tensor("iota8", [P, K8, P], mybir.dt.bfloat16,
                           kind="ExternalInput")
    if with_pool:
        gidc = nc.dram_tensor("gidc", [P, NT], mybir.dt.float32,
                              kind="ExternalInput")
        iotaB = nc.dram_tensor("iotaB", [P, P], mybir.dt.bfloat16,
                               kind="ExternalInput")
        poolT = nc.dram_tensor("poolT", [P, N_GRAPHS], mybir.dt.float32,
                               kind="ExternalOutput")
    else:
        hout = nc.dram_tensor("hout", [OWN, D], mybir.dt.bfloat16,
                              kind="ExternalOutput")
    zw = [nc.dram_tensor(f"z{w}", [WIN, D], mybir.dt.bfloat16) for w in range(NW)]

    with tile.TileContext(nc) as tc:
        with tc.tile_pool(name="const", bufs=1) as cp, \
             tc.tile_pool(name="hblk", bufs=2) as hp, \
             tc.tile_pool(name="zst", bufs=3) as zp, \
             tc.tile_pool(name="zps", bufs=2, space="PSUM") as zps, \
             tc.tile_pool(name="m0", bufs=2) as mp0, \
             tc.tile_pool(name="m1", bufs=2) as mp1, \
             tc.tile_pool(name="m2", bufs=2) as mp2, \
             tc.tile_pool(name="m3", bufs=2) as mp3, \
             tc.tile_pool(name="sbat", bufs=2) as sbp, \
             tc.tile_pool(name="sg", bufs=4) as sgp, \
             tc.tile_pool(name="cps", bufs=2, space="PSUM") as cpsp, \
             tc.tile_pool(name="pps", bufs=1, space="PSUM") as ppsp, \
             tc.tile_pool(name="dph", bufs=3) as dp:
            mpools = [mp0, mp1, mp2, mp3]
            W_sb = cp.tile([D, D], mybir.dt.bfloat16, tag="W")
            nc.sync.dma_start(out=W_sb[:], in_=W[:])
            brep_sb = cp.tile([P, D], mybir.dt.float32, tag="brep")
            nc.sync.dma_start(out=brep_sb[:], in_=brep[:])
            ndst_sb = cp.tile([P, NT], mybir.dt.float32, tag="ndst")
            nc.sync.dma_start(out=ndst_sb[:], in_=ndstc[:])
            dl_sb = cp.tile([P, njobs_pad], mybir.dt.float32, tag="dl")
            nc.sync.dma_start(out=dl_sb[:], in_=dl[:])
            if fuse_nsl:
                nsl_sb = cp.tile([P, njobs_pad], mybir.dt.float32, tag="nsl")
                nc.sync.dma_start(out=nsl_sb[:], in_=nsl[:])
            if fold_nsrc:
                nsrc_sb = cp.tile([P, NT], mybir.dt.float32, tag="nsrc")
                nc.sync.dma_start(out=nsrc_sb[:], in_=nsrcc[:])
            idx_sb = cp.tile([P, IDXC], mybir.dt.int16, tag="idx")
            nc.sync.dma_start(out=idx_sb[:], in_=idx16[:])
            iota_sb = cp.tile([P, P], mybir.dt.bfloat16, tag="iota")
            nc.sync.dma_start(out=iota_sb[:], in_=iota[:])
            iota8_sb = cp.tile([P, K8, P], mybir.dt.bfloat16, tag="iota8")
            nc.sync.dma_start(out=iota8_sb[:], in_=iota8[:])
            if with_pool:
                gid_sb = cp.tile([P, NT], mybir.dt.float32, tag="gid")
                nc.sync.dma_start(out=gid_sb[:], in_=gidc[:])
                iotaB_sb = cp.tile([P, P], mybir.dt.bfloat16, tag="iotaB")
                nc.sync.dma_start(out=iotaB_sb[:], in_=iotaB[:])
                pool_ps0 = ppsp.tile([P, P], mybir.dt.float32, tag="pool0")
                pool_ps1 = ppsp.tile([P, P], mybir.dt.float32, tag="pool1")
                pool_ps = [pool_ps0, pool_ps1]

            # ---- phase A: z = h @ W  (bf16), per window ----
            # z stored partition-major: window node n -> z row
            # (n % 128) * 196 + n // 128, so each zst store is one
            # contiguous ZB*256B descriptor per partition.
            ZB = 8   # chunks per psum group / staging store
            HB = 28  # chunks per hT staging block
            WCH = WIN // P  # 196 chunks per window
            for w in range(NW):
                vw = zw[w][:].rearrange("(p c) f -> p c f", c=WCH)
                for blk in range(WCH // HB):
                    c0 = blk * HB
                    hT_sb = hp.tile([P, HB * P], mybir.dt.bfloat16, tag="h")
                    nc.sync.dma_start(
                        out=hT_sb[:],
                        in_=hT[:, w * WIN + c0 * P:w * WIN + (c0 + HB) * P])
                    for g0 in range(0, HB, ZB):
                        zb = min(ZB, HB - g0)
                        ps = zps.tile([P, ZB, D], mybir.dt.float32, tag="zps")
                        for k in range(zb):
                            nc.tensor.matmul(
                                out=ps[:, k, :],
                                lhsT=hT_sb[:, (g0 + k) * P:(g0 + k + 1) * P],
                                rhs=W_sb[:], start=True, stop=True)
                        zst = zp.tile([P, ZB, D], mybir.dt.bfloat16, tag="zst")
                        nc.scalar.activation(
                            out=zst[:, :zb, :], in_=ps[:, :zb, :],
                            func=mybir.ActivationFunctionType.Copy)
                        nc.sync.dma_start(
                            out=vw[:, c0 + g0:c0 + g0 + zb, :],
                            in_=zst[:, :zb, :])

            # ---- phase B: gathers (prep + trigger), round-robin across
            # windows/queues; Tile defers the z RAW dep to the trigger and
            # makes consumers wait on the prep's DMA-completion tick. ----
            msg_tiles = {w: [] for w in range(NW)}
            woff = [0] * NW
            for (w, n) in calls:
                mt = mpools[w].tile([P, n // P, D], mybir.dt.bfloat16,
                                    tag=f"msg{w}")
                so = _stream_slot_off(meta, w) + woff[w]
                nc.gpsimd.dma_gather(
                    mt[:], zw[w][:], idx_sb[:, so // 16:(so + n) // 16],
                    n, n, D, queue_num=w, single_packet=False)
                msg_tiles[w].append(mt)
                woff[w] += n

            # ---- phase C+D: tile-major segmented reduce with prebuilt S ----
            sbats = []

            def ensure_sbat(ji):
                b = ji // SBATCH
                while len(sbats) <= b:
                    j0 = len(sbats) * SBATCH
                    SB = sbp.tile([P, SBATCH, P], mybir.dt.bfloat16, tag="SB")
                    if fuse_nsl:
                        for k in range(SBATCH):
                            nc.vector.tensor_scalar(
                                out=SB[:, k, :], in0=iota_sb[:],
                                scalar1=dl_sb[:, j0 + k:j0 + k + 1],
                                scalar2=nsl_sb[:, j0 + k:j0 + k + 1],
                                op0=mybir.AluOpType.is_equal,
                                op1=mybir.AluOpType.mult)
                    else:
                        for k in range(0, SBATCH, K8):
                            nc.vector.tensor_tensor(
                                out=SB[:, k:k + K8, :],
                                in0=dl_sb[:, j0 + k:j0 + k + K8]
                                .to_broadcast([P, K8, P]),
                                in1=iota8_sb[:],
                                op=mybir.AluOpType.is_equal)
                    sbats.append(SB)
                return sbats[b]

            ji = 0
            for t in range(NT):
                nj = tile_njobs[t]
                ps = cpsp.tile([P, D], mybir.dt.float32, tag="cps")
                for u in range(nj):
                    w, cw, gc = jobs[ji]
                    SB = ensure_sbat(ji)
                    ci, slot = chunk_map[(w, cw)]
                    nc.tensor.matmul(
                        out=ps[:], lhsT=SB[:, ji % SBATCH, :],
                        rhs=msg_tiles[w][ci][:, slot, :],
                        start=(u == 0), stop=(u == nj - 1))
                    ji += 1
                hn = dp.tile([P, D], mybir.dt.bfloat16, tag="hn")
                nc.vector.scalar_tensor_tensor(
                    out=hn[:], in0=ps[:], scalar=ndst_sb[:, t:t + 1],
                    in1=brep_sb[:], op0=mybir.AluOpType.mult,
                    op1=mybir.AluOpType.add)
                if fold_nsrc:
                    nc.scalar.activation(
                        out=hn[:], in_=hn[:],
                        func=mybir.ActivationFunctionType.Relu,
                        scale=nsrc_sb[:, t:t + 1])
                else:
                    nc.scalar.activation(
                        out=hn[:], in_=hn[:],
                        func=mybir.ActivationFunctionType.Relu)
                if with_pool:
                    for b, io_sb in enumerate((iota_sb, iotaB_sb)):
                        Sg = sgp.tile([P, P], mybir.dt.bfloat16, tag="Sg")
                        nc.vector.tensor_single_scalar(
                            out=Sg[:], in_=io_sb[:],
                            scalar=gid_sb[:, t:t + 1],
                            op=mybir.AluOpType.is_equal)
                        nc.tensor.matmul(
                            out=pool_ps[b][:], lhsT=hn[:], rhs=Sg[:],
                            start=(t == 0), stop=(t == NT - 1))
                else:
                    nc.sync.dma_start(out=hout[t * P:(t + 1) * P, :], in_=hn[:])

            if with_pool:
                pool_sb = dp.tile([P, 2, P], mybir.dt.float32, tag="poolsb")
                for b in range(2):
                    nc.vector.tensor_copy(out=pool_sb[:, b, :], in_=pool_ps[b][:])
                nc.sync.dma_start(
                    out=poolT[:], in_=pool_sb[:].rearrange("p b g -> p (b g)"))
    nc.compile()
    return nc


def _build_tail():
    nc = bacc.Bacc("TRN2", num_devices=NC, num_swdge_queues=1)
    pall = nc.dram_tensor("pall", [NC * P, N_GRAPHS], mybir.dt.float32,
                          kind="ExternalInput")
    invc = nc.dram_tensor("invc", [P, N_GRAPHS], mybir.dt.float32,
                          kind="ExternalInput")
    Wf0 = nc.dram_tensor("Wf0", [128, 256], mybir.dt.float32, kind="ExternalInput")
    bf0 = nc.dram_tensor("bf0", [256, 1], mybir.dt.float32, kind="ExternalInput")
    Wf1 = nc.dram_tensor("Wf1", [256, 256], mybir.dt.float32, kind="ExternalInput")
    bf1 = nc.dram_tensor("bf1", [256, 1], mybir.dt.float32, kind="ExternalInput")
    Wout = nc.dram_tensor("Wout", [256, 8], mybir.dt.float32, kind="ExternalInput")
    bout = nc.dram_tensor("bout", [8, 1], mybir.dt.float32, kind="ExternalInput")
    outT = nc.dram_tensor("outT", [8, N_GRAPHS], mybir.dt.float32,
                          kind="ExternalOutput")

    with tile.TileContext(nc) as tc:
        with tc.tile_pool(name="c", bufs=1) as cp, \
             tc.tile_pool(name="ps", bufs=2, space="PSUM") as psp, \
             tc.tile_pool(name="mlp", bufs=1) as mlp:
            pal = cp.tile([P, NC, N_GRAPHS], mybir.dt.float32, tag="pal")
            nc.sync.dma_start(
                out=pal[:], in_=pall[:].rearrange("(c p) g -> p c g", p=P))
            ic_sb = cp.tile([P, N_GRAPHS], mybir.dt.float32, tag="ic")
            nc.sync.dma_start(out=ic_sb[:], in_=invc[:])
            w0 = cp.tile([128, 256], mybir.dt.float32, tag="w0")
            nc.sync.dma_start(out=w0[:], in_=Wf0[:])
            w1 = cp.tile([128, 2, 256], mybir.dt.float32, tag="w1")
            nc.sync.dma_start(out=w1[:], in_=Wf1[:].rearrange("(b k) o -> k b o", b=2))
            wo = cp.tile([128, 2, 8], mybir.dt.float32, tag="wo")
            nc.sync.dma_start(out=wo[:], in_=Wout[:].rearrange("(b k) o -> k b o", b=2))
            b0 = cp.tile([128, 2], mybir.dt.float32, tag="b0")
            nc.sync.dma_start(out=b0[:], in_=bf0[:].rearrange("(b k) o -> k (b o)", b=2))
            b1 = cp.tile([128, 2], mybir.dt.float32, tag="b1")
            nc.sync.dma_start(out=b1[:], in_=bf1[:].rearrange("(b k) o -> k (b o)", b=2))
            bo = cp.tile([8, 1], mybir.dt.float32, tag="bo")
            nc.sync.dma_start(out=bo[:], in_=bout[:])

            acc = mlp.tile([P, N_GRAPHS], mybir.dt.float32, tag="acc")
            nc.vector.tensor_copy(out=acc[:], in_=pal[:, 0, :])
            for c in range(1, NC):
                nc.vector.tensor_add(out=acc[:], in0=acc[:], in1=pal[:, c, :])
            hgT = mlp.tile([P, N_GRAPHS], mybir.dt.float32, tag="hgT")
            nc.vector.tensor_tensor(out=hgT[:], in0=acc[:], in1=ic_sb[:],
                                    op=mybir.AluOpType.mult)

            a1_0 = mlp.tile([P, N_GRAPHS], mybir.dt.float32, tag="a1_0")
            a1_1 = mlp.tile([P, N_GRAPHS], mybir.dt.float32, tag="a1_1")
            a1 = [a1_0, a1_1]
            for ob in range(2):
                ps = psp.tile([P, N_GRAPHS], mybir.dt.float32, tag="mps")
                nc.tensor.matmul(out=ps[:], lhsT=w0[:, ob * 128:(ob + 1) * 128],
                                 rhs=hgT[:], start=True, stop=True)
                nc.vector.tensor_scalar_add(
                    out=a1[ob][:], in0=ps[:], scalar1=b0[:, ob:ob + 1])
                nc.scalar.activation(out=a1[ob][:], in_=a1[ob][:],
                                     func=mybir.ActivationFunctionType.Relu)
            a2_0 = mlp.tile([P, N_GRAPHS], mybir.dt.float32, tag="a2_0")
            a2_1 = mlp.tile([P, N_GRAPHS], mybir.dt.float32, tag="a2_1")
            a2 = [a2_0, a2_1]
            for ob in range(2):
                ps = psp.tile([P, N_GRAPHS], mybir.dt.float32, tag="mps")
                for ib in range(2):
                    nc.tensor.matmul(out=ps[:],
                                     lhsT=w1[:, ib, ob * 128:(ob + 1) * 128],
                                     rhs=a1[ib][:],
                                     start=(ib == 0), stop=(ib == 1))
                nc.vector.tensor_scalar_add(
                    out=a2[ob][:], in0=ps[:], scalar1=b1[:, ob:ob + 1])
                nc.scalar.activation(out=a2[ob][:], in_=a2[ob][:],
                                     func=mybir.ActivationFunctionType.Relu)
            ps = psp.tile([8, N_GRAPHS], mybir.dt.float32, tag="ops")
            for ib in range(2):
                nc.tensor.matmul(out=ps[:], lhsT=wo[:, ib, :], rhs=a2[ib][:],
                                 start=(ib == 0), stop=(ib == 1))
            oT = mlp.tile([8, N_GRAPHS], mybir.dt.float32, tag="oT")
            nc.vector.tensor_scalar_add(out=oT[:], in0=ps[:], scalar1=bo[:])
            nc.sync.dma_start(out=outT[:], in_=oT[:])
    nc.compile()
    return nc


def kernel(x, edge_src, edge_dst, node2graph,
           Wg0, bg0, Wg1, bg1, Wg2, bg2,
           Wf0, bf0, Wf1, bf1, Wout, bout):
    global LAST_EXEC_NS
    LAST_EXEC_NS = []
    per_core, meta, inv_cnt = _prep(edge_src, edge_dst, node2graph)

    trace = os.environ.get("GNN_TRACE", "0") == "1"

    def run(nc, in_maps):
        res = run_bass_kernel_spmd(nc, in_maps, core_ids=list(range(NC)),
                                   trace=trace)
        if res.exec_time_ns:
            LAST_EXEC_NS.append(res.exec_time_ns)
        return res

    iota128 = np.tile(np.arange(P, dtype=np.float32), (P, 1)).astype(
        ml_dtypes.bfloat16)
    iotaB128 = (np.tile(np.arange(P, dtype=np.float32), (P, 1)) + 128).astype(
        ml_dtypes.bfloat16)
    iota8 = np.tile(np.arange(P, dtype=np.float32), (P, 8)).astype(
        ml_dtypes.bfloat16).reshape(P, 8, P)

    conv1 = _build_conv(meta, fuse_nsl=True, fold_nsrc=True, with_pool=False)
    conv2 = _build_conv(meta, fuse_nsl=False, fold_nsrc=True, with_pool=False)
    conv3 = _build_conv(meta, fuse_nsl=False, fold_nsrc=False, with_pool=True)
    tailk = _build_tail()

    hT_pad = np.zeros((P, PADN), ml_dtypes.bfloat16)
    hT_pad[:, :N_NODES] = np.asarray(x, np.float32).T.astype(ml_dtypes.bfloat16)
    for li, (conv, Wl, bl) in enumerate(((conv1, Wg0, bg0), (conv2, Wg1, bg1),
                                         (conv3, Wg2, bg2))):
        in_maps = []
        for c in range(NC):
            im = dict(
                hT=hT_pad, W=np.asarray(Wl, np.float32).astype(ml_dtypes.bfloat16),
                brep=np.tile(np.asarray(bl, np.float32), (P, 1)),
                ndstc=per_core[c]["ndstc"], dl=per_core[c]["dl"],
                idx16=per_core[c]["idx16"],
                iota=iota128, iota8=iota8)
            if li == 0:
                im["nsl"] = per_core[c]["nsl"]
            if li < 2:
                im["nsrcc"] = per_core[c]["nsrcc"]
            else:
                im["gidc"] = per_core[c]["gidc"]
                im["iotaB"] = iotaB128
            in_maps.append(im)
        res = run(conv, in_maps)
        if li < 2:
            hT_pad = np.zeros((P, PADN), ml_dtypes.bfloat16)
            for c in range(NC):
                lo, hi = c * OWN, min((c + 1) * OWN, N_NODES)
                hT_pad[:, lo:hi] = res.results[c]["hout"][:hi - lo].T

    pall = np.concatenate([res.results[c]["poolT"] for c in range(NC)], axis=0)
    im = dict(pall=pall,
              invc=np.tile(inv_cnt, (P, 1)),
              Wf0=np.asarray(Wf0, np.float32),
              bf0=np.asarray(bf0, np.float32).reshape(256, 1),
              Wf1=np.asarray(Wf1, np.float32),
              bf1=np.asarray(bf1, np.float32).reshape(256, 1),
              Wout=np.asarray(Wout, np.float32),
              bout=np.asarray(bout, np.float32).reshape(8, 1))
    res = run(tailk, [dict(im) for _ in range(NC)])
    return np.ascontiguousarray(res.results[0]["outT"].T)
